# revision 54
# baseline (speedup 1.0000x reference)
"""Distributed Trainium2 (Bass/Tile) kernel for the AdaMEOW GNN loss.

Sharding: target-node dim N row-sharded across 8 cores (128 rows each);
neighbor dim M sharded (512 each) for the neighbor-feature MLPs, combined
with one fp8 ReduceScatter (counts are computed locally from row-shards of
nei).  The z_coarse (mean-adjacency GCN) chain runs entirely inside the
ReduceScatter window: a tiny y1-mean AllGather fires before the RS, each
core then runs the full-N mean convolution locally (host-precomputed
mean-adjacency), so only three collectives remain after the RS (fine y1,
fine y2, attention partials) plus the early z_coarse AllGather.
The [N,N,E] InfoNCE pair tensor is never materialized: the pair-MLP is
fused as w[i,j] = sigmoid(sum_h tanh(A[i,h]+B[j,h])*m2[h]+b2), with
sigmoid computed via tanh to stay on one activation table; all l2-norm
rsqrts use a table-free quake-style Newton iteration on the DVE.
"""

import os

import ml_dtypes
import numpy as np

import concourse.bass as bass
import concourse.mybir as mybir
import concourse.tile as tile
from concourse import bacc
from concourse.bass_utils import run_bass_kernel_spmd

FP = mybir.dt.float32
BF = mybir.dt.bfloat16
F8 = mybir.dt.float8e4
NPBF = ml_dtypes.bfloat16
NPF8 = ml_dtypes.float8_e4m3
AF = mybir.ActivationFunctionType
ALU = mybir.AluOpType
DR = mybir.MatmulPerfMode.DoubleRow

N, M, D0, D1, H, E = 1024, 4096, 1024, 512, 512, 64
C = 8            # cores
NL = N // C      # 128 local target nodes
ML = M // C      # 512 local neighbor nodes
P = 128
HK = H // P      # 4
D0K = D0 // P    # 8
MLK = ML // P    # 4
NB = N // P      # 8 node blocks
TAU = 0.5
RG = [list(range(C))]


def _build():
    nc = bacc.Bacc("TRN2", num_devices=C)

    def din(name, shape, dt=BF):
        return nc.declare_dram_parameter(name, list(shape), dt, isOutput=False)

    # per-core sharded inputs (host pre-arranged to final SBUF layouts)
    feat1T = din("feat1T", (P, MLK * ML), F8)     # [p, mlk, ML]
    feat2T = din("feat2T", (P, MLK * ML), F8)
    nei0T = din("nei0T", (P, MLK * N), F8)        # [p, mlk, N]
    nei1T = din("nei1T", (P, MLK * N), F8)
    nei0R = din("nei0R", (P, M), BF)              # local rows of nei0
    nei1R = din("nei1R", (P, M), BF)
    f0m = din("f0m", (P, D0K * 2 * NL), BF)       # [p, k, tar|mask]
    adj0T = din("adj0T", (P, NB * NL), F8)
    adj1T = din("adj1T", (P, NB * NL), F8)
    madj0T = din("madj0T", (P, NB * NL), F8)
    madj1T = din("madj1T", (P, NB * NL), F8)
    mnadjTf = din("mnadjTf", (P, NB * N), F8)     # full (adj0+adj1).T
    mnadjTl = din("mnadjTl", (P, NB * NL), F8)    # local (adj0+adj1).T
    # replicated weights
    fc0_w = din("fc0_w", (P, D0K * H), BF)
    fc1_w = din("fc1_w", (P, MLK * H), F8)
    fc2_w = din("fc2_w", (P, MLK * H), F8)
    agg0_w = din("agg0_w", (P, HK * H), F8)
    agg1_w = din("agg1_w", (P, HK * H), F8)
    gcn_w1 = din("gcn_w1", (P, HK * E), F8)
    gcn_w2 = din("gcn_w2", (E, E), BF)
    att_w = din("att_w", (E, E), BF)
    proj_w = din("proj_w", (E, E), BF)
    mlp1_w = din("mlp1_w", (E, 16), BF)
    sel16 = din("sel16", (16, 16 * P), BF)        # eye16 (x) ones(1,P)
    eye128 = din("eye128", (P, P), BF)
    # small aux tensors
    fc0_b = din("fc0_b", (P, HK), FP)             # [p, hc] feature-partition
    fc1_b = din("fc1_b", (1, H), BF)              # row (for psum bias init)
    fc2_b = din("fc2_b", (1, H), BF)
    gcn_b1 = din("gcn_b1", (P, 1), FP)            # tiled x2 -> [128,1]
    gcn_b2 = din("gcn_b2", (P, 1), FP)
    att_b = din("att_b", (E, 1), FP)
    att_vec = din("att_vec", (E, 1), BF)
    proj_b = din("proj_b", (E, 1), FP)
    mlp1_b = din("mlp1_b", (1, 16), FP)
    mlp2_w = din("mlp2_w", (1, 16), FP)           # mlp2_w.T
    mlp2_b = din("mlp2_b", (1, 1), FP)

    out_ext = nc.declare_dram_parameter("out", [NL, 2], FP, isOutput=True)

    # collective bounce buffers
    ag0_in = nc.dram_tensor("ag0_in", [1, P], BF)
    ag0_out = nc.dram_tensor("ag0_out", [C, P], BF, addr_space="Shared")
    agm_in = nc.dram_tensor("agm_in", [NL, E], F8)
    agm_out = nc.dram_tensor("agm_out", [N, E], F8, addr_space="Shared")
    rs_in = nc.dram_tensor("rs_in", [NB, P, 2 * HK, P], F8)
    rs_out = nc.dram_tensor("rs_out", [P, 2 * HK, P], F8)
    ag3_in = nc.dram_tensor("ag3_in", [E, P], BF)
    ag3_out = nc.dram_tensor("ag3_out", [C * E, P], BF, addr_space="Shared")
    ag1_in = nc.dram_tensor("ag1_in", [NL, 4 * E], F8)
    ag1_out = nc.dram_tensor("ag1_out", [N, 4 * E], F8, addr_space="Shared")
    ag2_in = nc.dram_tensor("ag2_in", [NL, 4 * E], F8)
    ag2_out = nc.dram_tensor("ag2_out", [N, 4 * E], F8, addr_space="Shared")
    ag3a_in = nc.dram_tensor("ag3a_in", [1, P], BF)
    ag3a_out = nc.dram_tensor("ag3a_out", [C, P], BF, addr_space="Shared")

    with tile.TileContext(nc) as tc:
        with (
            tc.tile_pool(name="pers", bufs=1) as pers,
            tc.tile_pool(name="wkE", bufs=4) as wkE,
            tc.tile_pool(name="wkT", bufs=3) as wkT,
            tc.tile_pool(name="wkS", bufs=2) as wkS,
        ):
            def mk(pool, shape, name, dt=FP):
                return pool.tile(list(shape), dt, tag=name, name=name)

            def ld(pool, dram, shape, name, eng=None):
                t = mk(pool, shape, name, dt=dram.dtype)
                src = dram[:]
                if list(t.shape) != list(dram.shape):
                    src = src.rearrange("p (a b) -> p a b", a=t.shape[1])
                (eng or nc.sync).dma_start(t[:], src)
                return t

            def elu(ps_ap, ebias=0.0):
                """elu(x) = relu(x) + min(exp(x), 1) - 1; 2 ACT + 1 DVE."""
                sh = [ps_ap.shape[0], ps_ap.free_size()]
                e = wkE.tile(sh, BF, tag="elu_e", name="elu_e")
                r = wkE.tile(sh, BF, tag="elu_r", name="elu_r")
                nc.scalar.activation(e[:], ps_ap, AF.Exp, bias=ebias)
                nc.scalar.activation(r[:], ps_ap, AF.Relu, bias=ebias)
                q = wkE.tile(sh, BF, tag="elu_q", name="elu_q")
                nc.vector.tensor_scalar(
                    out=q[:], in0=e[:], scalar1=1.0, scalar2=-1.0,
                    op0=ALU.min, op1=ALU.add)
                return q, r

            def rsqrt_row(ps_ap, nl, tag):
                """Table-free rsqrt of a [1, nl] psum row (quake seed +
                one Newton step on DVE); returns bf16 [1, nl] tile."""
                x = wkS.tile([1, nl], FP, tag="rsq_x", name=tag + "x")
                nc.vector.tensor_scalar_max(x[:], ps_ap, 1e-24)
                sh = wkS.tile([1, nl], FP, tag="rsq_s", name=tag + "s")
                nc.vector.tensor_scalar(
                    out=sh[:].bitcast(mybir.dt.uint32),
                    in0=x[:].bitcast(mybir.dt.uint32),
                    scalar1=1, scalar2=0,
                    op0=ALU.logical_shift_right, op1=ALU.bitwise_or)
                cmagic = wkS.tile([1, nl], FP, tag="rsq_c", name=tag + "c")
                nc.vector.memset(cmagic[:].bitcast(mybir.dt.uint32),
                                 0x5f3759df)
                y = wkS.tile([1, nl], FP, tag="rsq_y", name=tag + "y")
                nc.vector.tensor_tensor(
                    out=y[:].bitcast(mybir.dt.uint32),
                    in0=cmagic[:].bitcast(mybir.dt.uint32),
                    in1=sh[:].bitcast(mybir.dt.uint32),
                    op=ALU.subtract)
                # Newton step(s): y *= 1.5 - 0.5 x y^2
                for it in range(1):
                    t = wkS.tile([1, nl], FP, tag="rsq_t", name=tag + "t")
                    nc.vector.tensor_tensor(out=t[:], in0=y[:], in1=y[:],
                                            op=ALU.mult)
                    nc.vector.tensor_tensor(out=t[:], in0=t[:], in1=x[:],
                                            op=ALU.mult)
                    nc.vector.tensor_scalar(
                        out=t[:], in0=t[:], scalar1=-0.5, scalar2=1.5,
                        op0=ALU.mult, op1=ALU.add)
                    nc.vector.tensor_tensor(out=y[:], in0=y[:], in1=t[:],
                                            op=ALU.mult)
                yb = wkS.tile([1, nl], BF, tag="rsq_b", name=tag + "b")
                nc.vector.tensor_copy(yb[:], y[:])
                return yb

            # ---------------- persistent constants --------------------
            ones_row = mk(pers, (1, 512), "ones_row", BF)
            nc.vector.memset(ones_row[:], 1.0)
            ones_col = mk(pers, (P, 1), "ones_col", BF)
            nc.vector.memset(ones_col[:], 1.0)

            # dummy first collective: rings the doorbell immediately
            # so the cross-rank model barrier resolves early
            dummy = wkS.tile([1, P], BF, tag="dummy", name="dummy")
            nc.vector.memset(dummy[:], 0.0)
            nc.sync.dma_start(ag0_in[:], dummy[:])
            nc.gpsimd.collective_compute(
                "AllGather", ALU.bypass, replica_groups=RG,
                ins=[ag0_in[:].opt()], outs=[ag0_out[:].opt()])

            # ================= stage 1: fp8 MLPs + aggregation ========
            f0m_sb = ld(pers, f0m, (P, D0K, 2 * NL), "f0m", nc.sync)
            fc0w_sb = ld(pers, fc0_w, (P, D0K, H), "fc0w", nc.gpsimd)
            fc0b_sb = ld(pers, fc0_b, (P, HK), "fc0b", nc.scalar)
            gcnw1_sb = ld(pers, gcn_w1, (P, HK, E), "gcnw1", nc.scalar)
            feat1T_sb = ld(pers, feat1T, (P, MLK, ML), "feat1T", nc.gpsimd)
            fc1w_sb = ld(pers, fc1_w, (P, MLK, H), "fc1w", nc.sync)
            feat2T_sb = ld(pers, feat2T, (P, MLK, ML), "feat2T", nc.gpsimd)
            fc2w_sb = ld(pers, fc2_w, (P, MLK, H), "fc2w", nc.sync)
            nei0T_sb = ld(pers, nei0T, (P, MLK, N), "nei0T", nc.gpsimd)
            nei1T_sb = ld(pers, nei1T, (P, MLK, N), "nei1T", nc.sync)
            fc1b_sb = ld(pers, fc1_b, (1, H), "fc1b", nc.scalar)
            fc2b_sb = ld(pers, fc2_b, (1, H), "fc2b", nc.scalar)

            hnei_sb = [mk(pers, (P, MLK, H), "hnei0", F8),
                       mk(pers, (P, MLK, H), "hnei1", F8)]

            with tc.tile_pool(name="psA", bufs=3, space="PSUM") as psA:
                # ---- h[tar|mask]T + y1_mean first: feeds the early
                # y1_mean AllGather (second collective doorbell) ------
                hthm_sb = mk(pers, (P, HK, 4 * NL), "hthm", BF)
                for hc in range(HK):
                    ps = psA.tile([P, 2 * NL], FP, tag="psA", name="ps_ht")
                    for k in range(D0K):
                        nc.tensor.matmul(
                            ps[:], fc0w_sb[:, k, hc * P:(hc + 1) * P],
                            f0m_sb[:, k, :],
                            start=(k == 0), stop=(k == D0K - 1))
                    q, r = elu(ps[:], ebias=fc0b_sb[:, hc:hc + 1])
                    nc.vector.tensor_tensor(
                        out=hthm_sb[:, hc, 0:2 * NL], in0=q[:], in1=r[:],
                        op=ALU.add)
                htar8 = mk(pers, (P, HK, NL), "htar8", F8)
                nc.vector.tensor_copy(htar8[:], hthm_sb[:, :, 0:NL])
                psm = psA.tile([P, E], FP, tag="psA", name="ps_y1m")
                for kp in range(HK // 2):
                    nc.tensor.matmul(
                        psm[:], htar8[:, 2 * kp:2 * kp + 2, :],
                        gcnw1_sb[:, 2 * kp:2 * kp + 2, :],
                        start=(kp == 0), stop=(kp == HK // 2 - 1),
                        perf_mode=DR)
                stm = wkS.tile([NL, E], F8, tag="stm", name="stm")
                nc.vector.tensor_scalar_mul(stm[:], psm[:], 0.5)
                nc.sync.dma_start(agm_in[:], stm[:])
                for hc in range(HK):
                    nc.vector.tensor_copy(hthm_sb[:, hc, 2 * NL:4 * NL],
                                          hthm_sb[:, hc, 0:2 * NL])

                # ---- h_nei shards: elu(featX @ fcX_w + b) in fp8 -----
                for v, (fT, fw, fb) in enumerate(
                    [(feat1T_sb, fc1w_sb, fc1b_sb),
                     (feat2T_sb, fc2w_sb, fc2b_sb)]
                ):
                    for mc in range(MLK):
                        ps = psA.tile([P, H], FP, tag="psA", name="ps_hnei")
                        nc.tensor.matmul(ps[:], ones_row[:, 0:P], fb[:],
                                         start=True, stop=False)
                        for kp in range(MLK // 2):
                            nc.tensor.matmul(
                                ps[:],
                                fT[:, 2 * kp:2 * kp + 2, mc * P:(mc + 1) * P],
                                fw[:, 2 * kp:2 * kp + 2, :],
                                start=False, stop=(kp == MLK // 2 - 1),
                                perf_mode=DR)
                        q, r = elu(ps[:])
                        nc.vector.tensor_tensor(
                            out=hnei_sb[v][:, mc, :], in0=q[:], in1=r[:],
                            op=ALU.add)

                # ---- partial aggregation (feature-major, fp8) --------
                wq = [nc.sync, nc.scalar]
                for v, neiT in enumerate([nei0T_sb, nei1T_sb]):
                    for hc in range(HK):
                        for jh in range(2):
                            ps = psA.tile([P, 512], FP, tag="psA",
                                          name="ps_pr")
                            for kp in range(MLK // 2):
                                nc.tensor.matmul(
                                    ps[:],
                                    hnei_sb[v][:, 2 * kp:2 * kp + 2,
                                               hc * P:(hc + 1) * P],
                                    neiT[:, 2 * kp:2 * kp + 2,
                                         jh * 512:(jh + 1) * 512],
                                    start=(kp == 0),
                                    stop=(kp == MLK // 2 - 1),
                                    perf_mode=DR)
                            prs = wkE.tile([P, 512], F8, tag="prs",
                                           name="prs")
                            nc.vector.tensor_copy(prs[:], ps[:])
                            wq[(hc + jh) % 2].dma_start(
                                rs_in[jh * 4:(jh + 1) * 4, :,
                                      v * HK + hc:v * HK + hc + 1,
                                      :].rearrange(
                                          "b p one n -> p b (one n)"),
                                prs[:].rearrange("p (b n) -> p b n", b=4))

            # cc stream order: AG(y1_mean) -> RS -> AG(zc) -> AG1 ...
            nc.gpsimd.collective_compute(
                "AllGather", ALU.bypass, replica_groups=RG,
                ins=[agm_in[:].opt()], outs=[agm_out[:].opt()])
            nc.gpsimd.collective_compute(
                "ReduceScatter", ALU.add, replica_groups=RG,
                ins=[rs_in[:].opt()], outs=[rs_out[:].opt()])

            # ============ phase 1 (overlaps the ReduceScatter) ========
            with tc.tile_pool(name="psB", bufs=3, space="PSUM") as psB, \
                 tc.tile_pool(name="psS", bufs=4, space="PSUM") as psS:
                adj0T_sb = ld(pers, adj0T, (P, NB, NL), "adj0T", nc.sync)
                adj1T_sb = ld(pers, adj1T, (P, NB, NL), "adj1T", nc.scalar)
                madj0T_sb = ld(pers, madj0T, (P, NB, NL), "madj0T", nc.sync)
                madj1T_sb = ld(pers, madj1T, (P, NB, NL), "madj1T",
                               nc.scalar)
                mnadjTf_sb = ld(pers, mnadjTf, (P, NB, N), "mnadjTf",
                                nc.sync)
                mnadjTl_sb = ld(pers, mnadjTl, (P, NB, NL), "mnadjTl",
                                nc.scalar)
                agg0w_sb = ld(pers, agg0_w, (P, HK, H), "agg0w", nc.sync)
                agg1w_sb = ld(pers, agg1_w, (P, HK, H), "agg1w", nc.scalar)
                nei0R_sb = ld(pers, nei0R, (P, M), "nei0R", nc.sync)
                nei1R_sb = ld(pers, nei1R, (P, M), "nei1R", nc.scalar)
                gcnw2_sb = ld(pers, gcn_w2, (E, E), "gcnw2", nc.sync)
                attw_sb = ld(pers, att_w, (E, E), "attw", nc.sync)
                projw_sb = ld(pers, proj_w, (E, E), "projw", nc.sync)
                mlp1w_sb = ld(pers, mlp1_w, (E, 16), "mlp1w", nc.sync)
                sel_sb = ld(pers, sel16, (16, 16 * P), "sel", nc.scalar)
                eye_sb = ld(pers, eye128, (P, P), "eye", nc.scalar)
                gcnb1_sb = ld(pers, gcn_b1, (P, 1), "gcnb1", nc.sync)
                gcnb2_sb = ld(pers, gcn_b2, (P, 1), "gcnb2", nc.sync)
                attb_sb = ld(pers, att_b, (E, 1), "attb", nc.sync)
                attv_sb = ld(pers, att_vec, (E, 1), "attv", nc.sync)
                projb_sb = ld(pers, proj_b, (E, 1), "projb", nc.sync)
                b1bc16 = mk(pers, (P, 16), "b1bc16")
                nc.sync.dma_start(b1bc16[:], mlp1_b[:].to_broadcast((P, 16)))
                m2bc = mk(pers, (P, 16), "m2bc")
                nc.sync.dma_start(m2bc[:], mlp2_w[:].to_broadcast((P, 16)))
                b2h = mk(pers, (P, 1), "b2h")
                nc.sync.dma_start(b2h[:], mlp2_b[:].to_broadcast((P, 1)))
                nc.vector.tensor_scalar_mul(b2h[:], b2h[:], 0.5)

                # counts from local nei rows (exact, no collective)
                cnt = [mk(pers, (P, 1), "cnt0"), mk(pers, (P, 1), "cnt1")]
                nc.vector.reduce_sum(cnt[0][:], nei0R_sb[:],
                                     axis=mybir.AxisListType.X)
                nc.vector.reduce_sum(cnt[1][:], nei1R_sb[:],
                                     axis=mybir.AxisListType.X)
                rec4 = []
                for v in range(2):
                    cm = wkS.tile([P, 1], FP, tag="cm", name="cm")
                    nc.vector.tensor_scalar_max(cm[:], cnt[v][:], 1.0)
                    rc = wkS.tile([P, 1], FP, tag="rc", name="rc")
                    nc.vector.reciprocal(rc[:], cm[:])
                    rcb = wkS.tile([P, 1], BF, tag="rcb", name="rcb")
                    nc.vector.tensor_copy(rcb[:], rc[:])
                    pst = psS.tile([1, P], FP, tag="psS", name="ps_rT")
                    nc.tensor.matmul(pst[:], rcb[:], eye_sb[:])
                    rrow4 = wkS.tile([1, 4, P], BF, tag="rrow4",
                                     name="rrow4")
                    for t4 in range(4):
                        nc.vector.tensor_copy(rrow4[:, t4, :], pst[:])
                    psb = psB.tile([P, 4 * P], FP, tag="psB", name="ps_rbc")
                    nc.tensor.matmul(psb[:], ones_row[:, 0:P],
                                     rrow4[:].rearrange("o a b -> o (a b)"))
                    rb = mk(pers, (P, 4 * P), f"rec4_{v}", BF)
                    nc.vector.tensor_copy(rb[:], psb[:])
                    rec4.append(rb)

                # ---- z_coarse chain: full-N mean conv (in RS window) -
                y1m_sb = mk(pers, (P, NB, E), "y1mall", F8)
                nc.sync.dma_start(
                    y1m_sb[:], agm_out[:].rearrange("(b p) e -> p b e", p=P))
                hmT_sb = mk(pers, (E, N), "hmT", F8)
                for jh in range(2):
                    ps = psB.tile([P, 512], FP, tag="psB", name="ps_hm")
                    for bp in range(NB // 2):
                        nc.tensor.matmul(
                            ps[0:E, :], y1m_sb[:, 2 * bp:2 * bp + 2, :],
                            mnadjTf_sb[:, 2 * bp:2 * bp + 2,
                                       jh * 512:(jh + 1) * 512],
                            start=(bp == 0), stop=(bp == NB // 2 - 1),
                            perf_mode=DR)
                    nc.vector.tensor_scalar(
                        out=hmT_sb[:, jh * 512:(jh + 1) * 512],
                        in0=ps[0:E, :], scalar1=gcnb1_sb[0:E, :],
                        scalar2=0.0, op0=ALU.add, op1=ALU.max)
                # y2_mean (x0.5 for the mean-adj sum) [p, NB, E] fp8
                y2m_sb = mk(pers, (P, NB, E), "y2m", F8)
                for b in range(NB):
                    ps = psS.tile([P, E], FP, tag="psS", name="ps_y2m")
                    nc.tensor.matmul(ps[:], hmT_sb[:, b * P:(b + 1) * P],
                                     gcnw2_sb[:])
                    nc.vector.tensor_scalar_mul(y2m_sb[:, b, :], ps[:], 0.5)
                zT_sb = mk(pers, (E, 5, NL), "zT", BF)
                pszm = psS.tile([E, NL], FP, tag="psS", name="ps_zm")
                for bp in range(NB // 2):
                    nc.tensor.matmul(
                        pszm[:], y2m_sb[:, 2 * bp:2 * bp + 2, :],
                        mnadjTl_sb[:, 2 * bp:2 * bp + 2, :],
                        start=(bp == 0), stop=(bp == NB // 2 - 1),
                        perf_mode=DR)
                nc.vector.tensor_scalar_add(zT_sb[:, 0, :], pszm[:],
                                            gcnb2_sb[0:E, :])

                # proj z_coarse + colnorm -> AG(zc)
                psz = psB.tile([P, 512], FP, tag="psB", name="ps_pzc")
                nc.tensor.matmul(psz[0:E, 0:NL], projw_sb[:], zT_sb[:, 0, :])
                tfc = mk(pers, (E, NL), "tfc")
                nc.scalar.activation(tfc[:], psz[0:E, 0:NL], AF.Tanh,
                                     bias=projb_sb[:])
                sqc = wkS.tile([E, NL], BF, tag="sqc", name="sqc")
                nc.vector.tensor_mul(sqc[:], tfc[:], tfc[:])
                pss = psS.tile([1, NL], FP, tag="psS", name="ps_ssc")
                nc.tensor.matmul(pss[:], ones_col[0:E, :], sqc[:])
                rzc = rsqrt_row(pss[:], NL, "rzc")
                psbz = psS.tile([E, NL], FP, tag="psS", name="ps_nbz")
                nc.tensor.matmul(psbz[:], ones_row[:, 0:E], rzc[:])
                zcT_sb = mk(pers, (E, NL), "zcT", BF)
                nc.vector.tensor_mul(zcT_sb[:], tfc[:], psbz[:])
                nc.sync.dma_start(ag3_in[:], zcT_sb[:])
                nc.gpsimd.collective_compute(
                    "AllGather", ALU.bypass, replica_groups=RG,
                    ins=[ag3_in[:].opt()], outs=[ag3_out[:].opt()])

                # ================= post-RS: views + fine GCN ==========
                aggT_sb = mk(pers, (P, 2 * HK, NL), "aggT", F8)
                nc.sync.dma_start(aggT_sb[:], rs_out[:])
                aggS_sb = mk(pers, (P, 2 * HK, 2 * NL), "aggS", F8)
                for v in range(2):
                    for half in range(2):
                        nc.vector.tensor_tensor(
                            out=aggS_sb[:, v * HK:(v + 1) * HK,
                                        half * NL:(half + 1) * NL],
                            in0=aggT_sb[:, v * HK:(v + 1) * HK, :],
                            in1=rec4[v][:].rearrange(
                                "p (a b) -> p a b", a=HK),
                            op=ALU.mult)

                # both views + masks in one [P, 512] pass per h-chunk:
                # cols [v0tar | v0mask | v1tar | v1mask]
                xs4 = mk(pers, (P, HK, 4 * NL), "xs4", F8)
                for hc in range(HK):
                    ps = psB.tile([P, 4 * NL], FP, tag="psB", name="ps_x2")
                    for v, aggw in enumerate([agg0w_sb, agg1w_sb]):
                        half = ps[:, v * 2 * NL:(v + 1) * 2 * NL]
                        nc.tensor.matmul(
                            half, eye_sb[:],
                            hthm_sb[:, hc, v * 2 * NL:(v + 1) * 2 * NL],
                            start=True, stop=False)
                        for kp in range(HK // 2):
                            nc.tensor.matmul(
                                half,
                                aggw[:, 2 * kp:2 * kp + 2,
                                     hc * P:(hc + 1) * P],
                                aggS_sb[:, v * HK + 2 * kp:
                                        v * HK + 2 * kp + 2, :],
                                start=False, stop=(kp == HK // 2 - 1),
                                perf_mode=DR)
                    q, r = elu(ps[:])
                    nc.vector.tensor_tensor(
                        out=xs4[:, hc, :], in0=q[:], in1=r[:], op=ALU.add)

                # GCN layer-1 linear; st4a cols [v0, v1, m0, m1]
                st4a = mk(pers, (NL, 4, E), "st4a", F8)
                for c0, slot in [(0, 0), (2 * NL, 1), (NL, 2), (3 * NL, 3)]:
                    ps = psS.tile([P, E], FP, tag="psS", name="ps_y1")
                    for kp in range(HK // 2):
                        nc.tensor.matmul(
                            ps[:], xs4[:, 2 * kp:2 * kp + 2, c0:c0 + NL],
                            gcnw1_sb[:, 2 * kp:2 * kp + 2, :],
                            start=(kp == 0), stop=(kp == HK // 2 - 1),
                            perf_mode=DR)
                    nc.vector.tensor_copy(st4a[:, slot, :], ps[:])
                nc.sync.dma_start(
                    ag1_in[:].rearrange("n (g e) -> n g e", g=4), st4a[:])
                nc.gpsimd.collective_compute(
                    "AllGather", ALU.bypass, replica_groups=RG,
                    ins=[ag1_in[:].opt()], outs=[ag1_out[:].opt()])

                def conv_fine(y_sb, badd, relu, outs):
                    """4 fine graph convs; y_sb [P, NB, 4E] fp8 cols
                    [v0, v1, m0, m1]; outs: list of 4 (dst_ap)."""
                    pp = [psS.tile([E, NL], FP, tag="psS", name=f"pc{g}")
                          for g in range(4)]
                    adjs = [adj0T_sb, adj1T_sb, madj0T_sb, madj1T_sb]
                    for bp in range(NB // 2):
                        for g in range(4):
                            nc.tensor.matmul(
                                pp[g][:],
                                y_sb[:, 2 * bp:2 * bp + 2,
                                     g * E:(g + 1) * E],
                                adjs[g][:, 2 * bp:2 * bp + 2, :],
                                start=(bp == 0), stop=(bp == NB // 2 - 1),
                                perf_mode=DR)
                    op1 = ALU.max if relu else ALU.bypass
                    for g in range(4):
                        nc.vector.tensor_scalar(
                            out=outs[g], in0=pp[g][:],
                            scalar1=badd[0:E, :], scalar2=0.0,
                            op0=ALU.add, op1=op1)

                y1_sb = mk(pers, (P, NB, 4 * E), "y1", F8)
                nc.sync.dma_start(
                    y1_sb[:], ag1_out[:].rearrange("(b p) f -> p b f", p=P))
                h4_sb = mk(pers, (E, 4, NL), "h4", BF)
                conv_fine(y1_sb, gcnb1_sb, True,
                          [h4_sb[:, g, :] for g in range(4)])
                st4b = mk(pers, (NL, 4, E), "st4b", F8)
                for g in range(4):
                    ps = psS.tile([P, E], FP, tag="psS", name="ps_y2")
                    nc.tensor.matmul(ps[:], h4_sb[:, g, :], gcnw2_sb[:])
                    nc.vector.tensor_copy(st4b[:, g, :], ps[:])
                nc.sync.dma_start(
                    ag2_in[:].rearrange("n (g e) -> n g e", g=4), st4b[:])
                nc.gpsimd.collective_compute(
                    "AllGather", ALU.bypass, replica_groups=RG,
                    ins=[ag2_in[:].opt()], outs=[ag2_out[:].opt()])

                # zcall + BT as soon as AG(zc) lands
                zcall_sb = mk(pers, (E, C, P), "zcall", BF)
                zsrc = ag3_out[:].rearrange("(s p) n -> p s n", p=E)
                nc.sync.dma_start(zcall_sb[:, 0:4, :], zsrc[:, 0:4, :])
                nc.scalar.dma_start(zcall_sb[:, 4:8, :], zsrc[:, 4:8, :])
                BT_sb = mk(pers, (16, N), "BT", BF)
                for jh in range(2):
                    pbt = psB.tile([P, 512], FP, tag="psB", name="ps_BT")
                    nc.tensor.matmul(pbt[0:16, 0:512], mlp1w_sb[:],
                                     zcall_sb[:, jh * 4:(jh + 1) * 4, :])
                    nc.vector.tensor_copy(
                        BT_sb[:, jh * 512:(jh + 1) * 512], pbt[0:16, 0:512])

                y2_sb = mk(pers, (P, NB, 4 * E), "y2", F8)
                nc.sync.dma_start(
                    y2_sb[:], ag2_out[:].rearrange("(b p) f -> p b f", p=P))
                # conv2 -> zT slots [mean, v0, m0, v1, m1]
                conv_fine(y2_sb, gcnb2_sb, False,
                          [zT_sb[:, 1, :], zT_sb[:, 3, :],
                           zT_sb[:, 2, :], zT_sb[:, 4, :]])

                # ---- attention (scale-after-matmul) -> AG3a ----------
                z4 = zT_sb[:, 1:5, :]
                psa4 = psB.tile([P, 512], FP, tag="psB", name="ps_att4")
                nc.tensor.matmul(psa4[0:E, 0:4 * NL], attw_sb[:], z4)
                sq4 = wkS.tile([E, 4 * NL], BF, tag="sq4", name="sq4")
                nc.vector.tensor_mul(sq4[:], z4, z4)
                psn4 = psB.tile([P, 512], FP, tag="psB", name="ps_n4")
                nc.tensor.matmul(psn4[0:1, 0:4 * NL], ones_col[0:E, :],
                                 sq4[:])
                rn4 = rsqrt_row(psn4[0:1, 0:4 * NL], 4 * NL, "rn4")
                psb4 = psB.tile([P, 512], FP, tag="psB", name="ps_nb4")
                nc.tensor.matmul(psb4[0:E, 0:4 * NL], ones_row[:, 0:E],
                                 rn4[:])
                rn4bc = wkS.tile([E, 4 * NL], FP, tag="rn4bc", name="rn4bc")
                nc.vector.tensor_copy(rn4bc[:], psb4[0:E, 0:4 * NL])
                ta4in = wkS.tile([E, 4 * NL], FP, tag="ta4in", name="ta4in")
                nc.vector.tensor_mul(ta4in[:], psa4[0:E, 0:4 * NL],
                                     rn4bc[:])
                ta4 = wkS.tile([E, 4 * NL], BF, tag="ta4", name="ta4")
                nc.scalar.activation(ta4[:], ta4in[:], AF.Tanh,
                                     bias=attb_sb[:])
                pse = psB.tile([P, 512], FP, tag="psB", name="ps_e")
                nc.tensor.matmul(pse[0:1, 0:4 * NL], attv_sb[:], ta4[:])
                er4 = wkS.tile([1, 4], FP, tag="er4", name="er4")
                nc.vector.reduce_sum(
                    er4[:],
                    pse[0:1, 0:4 * NL].rearrange("o (v n) -> o v n", v=4),
                    axis=mybir.AxisListType.X)
                e_row = wkS.tile([1, P], BF, tag="e_row", name="e_row")
                nc.vector.memset(e_row[:], 0.0)
                nc.vector.tensor_scalar_mul(e_row[:, 0:4], er4[:], 1.0 / N)
                nc.sync.dma_start(ag3a_in[:], e_row[:])
                nc.gpsimd.collective_compute(
                    "AllGather", ALU.bypass, replica_groups=RG,
                    ins=[ag3a_in[:].opt()], outs=[ag3a_out[:].opt()])

                # hs + per-view projections (overlap the AG3a wait)
                hsT_sb = mk(pers, (E, 4, NL), "hsT", BF)
                nc.vector.tensor_mul(hsT_sb[:], z4, rn4bc[:])
                pj4 = psB.tile([P, 512], FP, tag="psB", name="ps_pj4")
                nc.tensor.matmul(pj4[0:E, 0:4 * NL], projw_sb[:],
                                 hsT_sb[:])
                pj4b = wkS.tile([E, 4 * NL], FP, tag="pj4b", name="pj4b")
                nc.vector.tensor_copy(pj4b[:], pj4[0:E, 0:4 * NL])

                # ---- softmax over views; z_fine; proj; A; diag -------
                e8_sb = wkS.tile([C, 4], BF, tag="e8", name="e8")
                nc.sync.dma_start(e8_sb[:], ag3a_out[:, 0:4])
                pse2 = psS.tile([1, 4], FP, tag="psS", name="ps_e2")
                nc.tensor.matmul(pse2[:], ones_col[0:C, :], e8_sb[:])
                ee = wkS.tile([1, 4], FP, tag="ee", name="ee")
                nc.scalar.activation(ee[:], pse2[:], AF.Exp)
                se = wkS.tile([1, 1], FP, tag="se", name="se")
                nc.vector.reduce_sum(se[:], ee[:], axis=mybir.AxisListType.X)
                nc.vector.reciprocal(se[:], se[:])
                beta_row = wkS.tile([1, 4], BF, tag="beta", name="beta")
                nc.vector.tensor_scalar_mul(beta_row[:], ee[:], se[:])
                psbb = psS.tile([E, 4], FP, tag="psS", name="ps_beta")
                nc.tensor.matmul(psbb[:], ones_row[:, 0:E], beta_row[:])
                beta_bc = wkS.tile([E, 4], FP, tag="beta_bc",
                                   name="beta_bc")
                nc.vector.tensor_copy(beta_bc[:], psbb[:])

                zfp = wkS.tile([E, NL], FP, tag="zfp", name="zfp")
                nc.vector.tensor_scalar(
                    out=zfp[:], in0=pj4b[:, 0:NL], scalar1=beta_bc[:, 0:1],
                    scalar2=0.0, op0=ALU.mult, op1=ALU.add)
                for v in range(1, 4):
                    nc.vector.scalar_tensor_tensor(
                        out=zfp[:], in0=pj4b[:, v * NL:(v + 1) * NL],
                        scalar=beta_bc[:, v:v + 1], in1=zfp[:],
                        op0=ALU.mult, op1=ALU.add)
                tf2 = mk(pers, (E, NL), "tf2", BF)
                nc.scalar.activation(tf2[:], zfp[:], AF.Tanh,
                                     bias=projb_sb[:])
                sqf = wkS.tile([E, NL], BF, tag="sqf", name="sqf")
                nc.vector.tensor_mul(sqf[:], tf2[:], tf2[:])
                pssf = psS.tile([1, NL], FP, tag="psS", name="ps_ssf")
                nc.tensor.matmul(pssf[:], ones_col[0:E, :], sqf[:])
                rzf = rsqrt_row(pssf[:], NL, "rzf")
                psbf = psS.tile([E, NL], FP, tag="psS", name="ps_nbf")
                nc.tensor.matmul(psbf[:], ones_row[:, 0:E], rzf[:])
                zfn_bf = mk(pers, (E, NL), "zfn_bf", BF)
                nc.vector.tensor_mul(zfn_bf[:], tf2[:], psbf[:])

                # A = zfn @ mlp1_w + b1  [NL, 16]
                psa2 = psS.tile([NL, 16], FP, tag="psS", name="ps_A")
                nc.tensor.matmul(psa2[:], zfn_bf[:], mlp1w_sb[:])
                A_sb = mk(pers, (NL, 16), "A")
                nc.vector.tensor_add(A_sb[:], psa2[:], b1bc16[:, 0:16])

                # diag = (zfn.zc)/tau
                prod = wkS.tile([E, NL], BF, tag="prod", name="prod")
                nc.vector.tensor_mul(prod[:], zfn_bf[:], zcT_sb[:])
                psd = psS.tile([NL, 1], FP, tag="psS", name="ps_diag")
                nc.tensor.matmul(psd[:], prod[:], ones_col[0:E, :])
                diag_sb = mk(pers, (NL, 1), "diag")
                nc.vector.tensor_scalar_mul(diag_sb[:], psd[:], 1.0 / TAU)

            # ================= InfoNCE tail (2-bank psum) =============
            with tc.tile_pool(name="psT", bufs=3, space="PSUM") as psT:
                # dots = exp((zfn.zc) / tau)  (exp/tanh table)
                psl = psT.tile([P, N], FP, tag="psT", name="ps_log")
                for jh in range(2):
                    nc.tensor.matmul(
                        psl[:, jh * 512:(jh + 1) * 512], zfn_bf[:],
                        zcall_sb[:, jh * 4:(jh + 1) * 4, :])
                dots_sb = mk(pers, (P, N), "dots", BF)
                nc.scalar.activation(dots_sb[:], psl[:], AF.Exp,
                                     scale=1.0 / TAU)

                # acc = sum_h tanh(A[:,h] + B[j,h]) * m2[h]
                accv = mk(pers, (P, N), "accv", BF)
                accg = mk(pers, (P, N), "accg", BF)
                for h in range(16):
                    psbt = psT.tile([P, N], FP, tag="psT", name="ps_bbc")
                    for jh in range(2):
                        nc.tensor.matmul(
                            psbt[:, jh * 512:(jh + 1) * 512],
                            sel_sb[:, h * P:(h + 1) * P],
                            BT_sb[:, jh * 512:(jh + 1) * 512])
                    th = wkT.tile([P, N], BF, tag="th", name="th")
                    nc.scalar.activation(th[:], psbt[:], AF.Tanh,
                                         bias=A_sb[:, h:h + 1])
                    acc = accg if h % 2 == 1 else accv
                    if h < 2:
                        nc.vector.tensor_scalar(
                            out=acc[:], in0=th[:], scalar1=m2bc[:, h:h + 1],
                            scalar2=0.0, op0=ALU.mult, op1=ALU.add)
                    else:
                        nc.vector.scalar_tensor_tensor(
                            out=acc[:], in0=th[:], scalar=m2bc[:, h:h + 1],
                            in1=acc[:], op0=ALU.mult, op1=ALU.add)
                nc.vector.tensor_add(accv[:], accv[:], accg[:])

                # sigmoid(x) = 0.5 + 0.5*tanh(x/2): stay on the exp table
                wt_sb = wkT.tile([P, N], BF, tag="wt", name="wt")
                nc.scalar.activation(wt_sb[:], accv[:], AF.Tanh,
                                     scale=0.5, bias=b2h[:])
                w_sb = mk(pers, (P, N), "w", BF)
                nc.vector.tensor_scalar(
                    out=w_sb[:], in0=wt_sb[:], scalar1=0.5, scalar2=0.5,
                    op0=ALU.mult, op1=ALU.add)

                # denom = sum_j dots * w
                denom_sb = wkS.tile([P, 1], FP, tag="denom", name="denom")
                scr = wkT.tile([P, N], BF, tag="scr", name="scr")
                nc.vector.scalar_tensor_tensor(
                    out=scr[:], in0=dots_sb[:], scalar=1.0, in1=w_sb[:],
                    op0=ALU.bypass, op1=ALU.mult, accum_out=denom_sb[:])

                outt = wkS.tile([NL, 2], FP, tag="outt", name="outt")
                nc.vector.tensor_copy(outt[:, 0:1], denom_sb[:])
                nc.vector.tensor_copy(outt[:, 1:2], diag_sb[:])
                nc.sync.dma_start(out_ext[:], outt[:])

    nc.finalize()
    return nc


_NC_CACHE = {}


def _get_nc():
    if "nc" not in _NC_CACHE:
        _NC_CACHE["nc"] = _build()
    return _NC_CACHE["nc"]


def _part3(x, p=128):
    """[(o p), f] row-major -> [p, o*f] (partition-inner layout)."""
    o = x.shape[0] // p
    return x.reshape(o, p, x.shape[1]).transpose(1, 0, 2).reshape(p, -1)


def kernel(**inputs):
    inp = {k: np.ascontiguousarray(np.asarray(v, dtype=np.float32))
           for k, v in inputs.items()}
    nc = _get_nc()

    def bf(x):
        return np.ascontiguousarray(x.astype(NPBF))

    def f8(x):
        return np.ascontiguousarray(x.astype(NPF8))

    rep = {}
    rep["fc0_w"] = bf(_part3(inp["fc0_w"]))
    rep["fc1_w"] = f8(_part3(inp["fc1_w"]))
    rep["fc2_w"] = f8(_part3(inp["fc2_w"]))
    rep["agg0_w"] = f8(_part3(inp["agg0_w"]))
    rep["agg1_w"] = f8(_part3(inp["agg1_w"]))
    rep["gcn_w1"] = f8(_part3(inp["gcn_w1"]))
    for k in ["gcn_w2", "att_w", "proj_w", "mlp1_w"]:
        rep[k] = bf(inp[k])
    rep["sel16"] = bf(
        np.kron(np.eye(16, dtype=np.float32), np.ones((1, P), np.float32)))
    rep["eye128"] = bf(np.eye(P, dtype=np.float32))
    rep["fc0_b"] = np.ascontiguousarray(
        inp["fc0_b"].reshape(HK, P).T)                     # [p, hc]
    rep["fc1_b"] = bf(inp["fc1_b"].reshape(1, H))
    rep["fc2_b"] = bf(inp["fc2_b"].reshape(1, H))
    rep["gcn_b1"] = np.ascontiguousarray(
        np.tile(inp["gcn_b1"].reshape(E), 2).reshape(P, 1))
    rep["gcn_b2"] = np.ascontiguousarray(
        np.tile(inp["gcn_b2"].reshape(E), 2).reshape(P, 1))
    for k in ["att_b", "proj_b"]:
        rep[k] = np.ascontiguousarray(inp[k].reshape(E, 1))
    rep["att_vec"] = bf(inp["att_vec"].reshape(E, 1))
    rep["mlp1_b"] = np.ascontiguousarray(inp["mlp1_b"].reshape(1, 16))
    rep["mlp2_w"] = np.ascontiguousarray(inp["mlp2_w"].reshape(16, 1).T)
    rep["mlp2_b"] = np.ascontiguousarray(inp["mlp2_b"].reshape(1, 1))

    mnadj = inp["adj0"] + inp["adj1"]
    in_maps = []
    for r in range(C):
        rs = slice(r * NL, (r + 1) * NL)
        ms = slice(r * ML, (r + 1) * ML)
        d = dict(rep)
        d["feat1T"] = f8(_part3(inp["feat1"][ms].T))
        d["feat2T"] = f8(_part3(inp["feat2"][ms].T))
        d["nei0T"] = f8(_part3(inp["nei0"][:, ms].T))
        d["nei1T"] = f8(_part3(inp["nei1"][:, ms].T))
        d["nei0R"] = bf(inp["nei0"][rs])
        d["nei1R"] = bf(inp["nei1"][rs])
        d["f0m"] = bf(_part3(np.concatenate(
            [inp["feat0"][rs].T, inp["mask_feat"][rs].T], axis=1)))
        d["adj0T"] = f8(_part3(inp["adj0"][rs].T))
        d["adj1T"] = f8(_part3(inp["adj1"][rs].T))
        d["madj0T"] = f8(_part3(inp["madj0"][rs].T))
        d["madj1T"] = f8(_part3(inp["madj1"][rs].T))
        d["mnadjTf"] = f8(_part3(mnadj.T))
        d["mnadjTl"] = f8(_part3(mnadj[rs].T))
        in_maps.append(d)

    trace = bool(int(os.environ.get("KERNEL_TRACE", "0")))
    res = run_bass_kernel_spmd(
        nc, in_maps, core_ids=list(range(C)), trace=trace)
    if trace:
        _NC_CACHE["exec_time_ns"] = res.exec_time_ns
        _NC_CACHE["trace"] = res.instructions_and_trace
    total = 0.0
    for r in range(C):
        o = np.asarray(res.results[r]["out"], dtype=np.float64)
        total += float(np.sum(np.log(o[:, 0]) - o[:, 1]))
    return np.float32(total / N)


# revision 55
# speedup vs baseline: 1.0410x; 1.0410x over previous
"""Distributed Trainium2 (Bass/Tile) kernel for the AdaMEOW GNN loss.

Sharding: target-node dim N row-sharded across 8 cores (128 rows each);
neighbor dim M sharded (512 each) for the neighbor-feature MLPs, combined
with one fp8 ReduceScatter (counts are computed locally from row-shards of
nei).  The z_coarse (mean-adjacency GCN) chain runs entirely inside the
ReduceScatter window: a tiny y1-mean AllGather fires before the RS, each
core then runs the full-N mean convolution locally (host-precomputed
mean-adjacency), so only three collectives remain after the RS (fine y1,
fine y2, attention partials) plus the early z_coarse AllGather.
The [N,N,E] InfoNCE pair tensor is never materialized: the pair-MLP is
fused as w[i,j] = sigmoid(sum_h tanh(A[i,h]+B[j,h])*m2[h]+b2), with
sigmoid computed via tanh to stay on one activation table; all l2-norm
rsqrts use a table-free quake-style Newton iteration on the DVE.
"""

import os

import ml_dtypes
import numpy as np

import concourse.bass as bass
import concourse.mybir as mybir
import concourse.tile as tile
from concourse import bacc
from concourse.bass_utils import run_bass_kernel_spmd

FP = mybir.dt.float32
BF = mybir.dt.bfloat16
F8 = mybir.dt.float8e4
NPBF = ml_dtypes.bfloat16
NPF8 = ml_dtypes.float8_e4m3
AF = mybir.ActivationFunctionType
ALU = mybir.AluOpType
DR = mybir.MatmulPerfMode.DoubleRow

N, M, D0, D1, H, E = 1024, 4096, 1024, 512, 512, 64
C = 8            # cores
NL = N // C      # 128 local target nodes
ML = M // C      # 512 local neighbor nodes
P = 128
HK = H // P      # 4
D0K = D0 // P    # 8
MLK = ML // P    # 4
NB = N // P      # 8 node blocks
TAU = 0.5
RG = [list(range(C))]


def _build():
    nc = bacc.Bacc("TRN2", num_devices=C)

    def din(name, shape, dt=BF):
        return nc.declare_dram_parameter(name, list(shape), dt, isOutput=False)

    # per-core sharded inputs (host pre-arranged to final SBUF layouts)
    feat1T = din("feat1T", (P, MLK * ML), F8)     # [p, mlk, ML]
    feat2T = din("feat2T", (P, MLK * ML), F8)
    nei0T = din("nei0T", (P, MLK * N), F8)        # [p, mlk, N]
    nei1T = din("nei1T", (P, MLK * N), F8)
    nei0R = din("nei0R", (P, M), BF)              # local rows of nei0
    nei1R = din("nei1R", (P, M), BF)
    f0m = din("f0m", (P, D0K * 2 * NL), BF)       # [p, k, tar|mask]
    adj0T = din("adj0T", (P, NB * NL), F8)
    adj1T = din("adj1T", (P, NB * NL), F8)
    madj0T = din("madj0T", (P, NB * NL), F8)
    madj1T = din("madj1T", (P, NB * NL), F8)
    mnadjTf = din("mnadjTf", (P, NB * N), F8)     # full (adj0+adj1).T
    mnadjTl = din("mnadjTl", (P, NB * NL), F8)    # local (adj0+adj1).T
    # replicated weights
    fc0_w = din("fc0_w", (P, D0K * H), BF)
    fc1_w = din("fc1_w", (P, MLK * H), F8)
    fc2_w = din("fc2_w", (P, MLK * H), F8)
    agg0_w = din("agg0_w", (P, HK * H), F8)
    agg1_w = din("agg1_w", (P, HK * H), F8)
    gcn_w1 = din("gcn_w1", (P, HK * E), F8)
    gcn_w2 = din("gcn_w2", (E, E), BF)
    att_w = din("att_w", (E, E), BF)
    proj_w = din("proj_w", (E, E), BF)
    mlp1_w = din("mlp1_w", (E, 16), BF)
    sel16 = din("sel16", (16, 16 * P), BF)        # eye16 (x) ones(1,P)
    eye128 = din("eye128", (P, P), BF)
    # small aux tensors
    fc0_b = din("fc0_b", (P, HK), FP)             # [p, hc] feature-partition
    fc1_b = din("fc1_b", (1, H), BF)              # row (for psum bias init)
    fc2_b = din("fc2_b", (1, H), BF)
    gcn_b1 = din("gcn_b1", (P, 1), FP)            # tiled x2 -> [128,1]
    gcn_b2 = din("gcn_b2", (P, 1), FP)
    att_b = din("att_b", (E, 1), FP)
    att_vec = din("att_vec", (E, 1), BF)
    proj_b = din("proj_b", (E, 1), FP)
    mlp1_b = din("mlp1_b", (1, 16), FP)
    mlp2_w = din("mlp2_w", (1, 16), FP)           # mlp2_w.T
    mlp2_b = din("mlp2_b", (1, 1), FP)

    out_ext = nc.declare_dram_parameter("out", [NL, 2], FP, isOutput=True)

    # collective bounce buffers
    ag0_in = nc.dram_tensor("ag0_in", [1, P], BF)
    ag0_out = nc.dram_tensor("ag0_out", [C, P], BF, addr_space="Shared")
    agm_in = nc.dram_tensor("agm_in", [NL, E], F8)
    agm_out = nc.dram_tensor("agm_out", [N, E], F8, addr_space="Shared")
    rs_in = nc.dram_tensor("rs_in", [NB, P, 2 * HK, P], F8)
    rs_out = nc.dram_tensor("rs_out", [P, 2 * HK, P], F8)
    ag3_in = nc.dram_tensor("ag3_in", [E, P], BF)
    ag3_out = nc.dram_tensor("ag3_out", [C * E, P], BF, addr_space="Shared")
    ag1_in = nc.dram_tensor("ag1_in", [NL, 4 * E], F8)
    ag1_out = nc.dram_tensor("ag1_out", [N, 4 * E], F8, addr_space="Shared")
    ag2_in = nc.dram_tensor("ag2_in", [NL, 4 * E], F8)
    ag2_out = nc.dram_tensor("ag2_out", [N, 4 * E], F8, addr_space="Shared")
    ag3a_in = nc.dram_tensor("ag3a_in", [1, P], BF)
    ag3a_out = nc.dram_tensor("ag3a_out", [C, P], BF, addr_space="Shared")

    with tile.TileContext(nc) as tc:
        with (
            tc.tile_pool(name="pers", bufs=1) as pers,
            tc.tile_pool(name="wkE", bufs=4) as wkE,
            tc.tile_pool(name="wkT", bufs=3) as wkT,
            tc.tile_pool(name="wkS", bufs=2) as wkS,
        ):
            def mk(pool, shape, name, dt=FP):
                return pool.tile(list(shape), dt, tag=name, name=name)

            def ld(pool, dram, shape, name, eng=None):
                t = mk(pool, shape, name, dt=dram.dtype)
                src = dram[:]
                if list(t.shape) != list(dram.shape):
                    src = src.rearrange("p (a b) -> p a b", a=t.shape[1])
                (eng or nc.sync).dma_start(t[:], src)
                return t

            def elu(ps_ap, ebias=0.0):
                """elu(x) = relu(x) + min(exp(x), 1) - 1; 2 ACT + 1 DVE."""
                sh = [ps_ap.shape[0], ps_ap.free_size()]
                e = wkE.tile(sh, BF, tag="elu_e", name="elu_e")
                r = wkE.tile(sh, BF, tag="elu_r", name="elu_r")
                nc.scalar.activation(e[:], ps_ap, AF.Exp, bias=ebias)
                nc.scalar.activation(r[:], ps_ap, AF.Relu, bias=ebias)
                q = wkE.tile(sh, BF, tag="elu_q", name="elu_q")
                nc.vector.tensor_scalar(
                    out=q[:], in0=e[:], scalar1=1.0, scalar2=-1.0,
                    op0=ALU.min, op1=ALU.add)
                return q, r

            def rsqrt_row(ps_ap, nl, tag):
                """Table-free rsqrt of a [1, nl] psum row (quake seed +
                one Newton step on DVE); returns bf16 [1, nl] tile."""
                x = wkS.tile([1, nl], FP, tag="rsq_x", name=tag + "x")
                nc.vector.tensor_scalar_max(x[:], ps_ap, 1e-24)
                sh = wkS.tile([1, nl], FP, tag="rsq_s", name=tag + "s")
                nc.vector.tensor_scalar(
                    out=sh[:].bitcast(mybir.dt.uint32),
                    in0=x[:].bitcast(mybir.dt.uint32),
                    scalar1=1, scalar2=0,
                    op0=ALU.logical_shift_right, op1=ALU.bitwise_or)
                cmagic = wkS.tile([1, nl], FP, tag="rsq_c", name=tag + "c")
                nc.vector.memset(cmagic[:].bitcast(mybir.dt.uint32),
                                 0x5f3759df)
                y = wkS.tile([1, nl], FP, tag="rsq_y", name=tag + "y")
                nc.vector.tensor_tensor(
                    out=y[:].bitcast(mybir.dt.uint32),
                    in0=cmagic[:].bitcast(mybir.dt.uint32),
                    in1=sh[:].bitcast(mybir.dt.uint32),
                    op=ALU.subtract)
                # Newton step(s): y *= 1.5 - 0.5 x y^2
                for it in range(1):
                    t = wkS.tile([1, nl], FP, tag="rsq_t", name=tag + "t")
                    nc.vector.tensor_tensor(out=t[:], in0=y[:], in1=y[:],
                                            op=ALU.mult)
                    nc.vector.tensor_tensor(out=t[:], in0=t[:], in1=x[:],
                                            op=ALU.mult)
                    nc.vector.tensor_scalar(
                        out=t[:], in0=t[:], scalar1=-0.5, scalar2=1.5,
                        op0=ALU.mult, op1=ALU.add)
                    nc.vector.tensor_tensor(out=y[:], in0=y[:], in1=t[:],
                                            op=ALU.mult)
                yb = wkS.tile([1, nl], BF, tag="rsq_b", name=tag + "b")
                nc.vector.tensor_copy(yb[:], y[:])
                return yb

            # ---------------- persistent constants --------------------
            ones_row = mk(pers, (1, 512), "ones_row", BF)
            nc.vector.memset(ones_row[:], 1.0)
            ones_col = mk(pers, (P, 1), "ones_col", BF)
            nc.vector.memset(ones_col[:], 1.0)

            # dummy first collective on uninitialized scratch: its trigger
            # has no input dependency, so the doorbell rings immediately
            # and the cross-rank model barrier resolves as early as the
            # launch skew allows
            nc.gpsimd.collective_compute(
                "AllGather", ALU.bypass, replica_groups=RG,
                ins=[ag0_in[:].opt()], outs=[ag0_out[:].opt()])

            # ================= stage 1: fp8 MLPs + aggregation ========
            f0m_sb = ld(pers, f0m, (P, D0K, 2 * NL), "f0m", nc.sync)
            fc0w_sb = ld(pers, fc0_w, (P, D0K, H), "fc0w", nc.gpsimd)
            fc0b_sb = ld(pers, fc0_b, (P, HK), "fc0b", nc.scalar)
            gcnw1_sb = ld(pers, gcn_w1, (P, HK, E), "gcnw1", nc.scalar)
            feat1T_sb = ld(pers, feat1T, (P, MLK, ML), "feat1T", nc.gpsimd)
            fc1w_sb = ld(pers, fc1_w, (P, MLK, H), "fc1w", nc.sync)
            feat2T_sb = ld(pers, feat2T, (P, MLK, ML), "feat2T", nc.gpsimd)
            fc2w_sb = ld(pers, fc2_w, (P, MLK, H), "fc2w", nc.sync)
            nei0T_sb = ld(pers, nei0T, (P, MLK, N), "nei0T", nc.gpsimd)
            nei1T_sb = ld(pers, nei1T, (P, MLK, N), "nei1T", nc.sync)
            fc1b_sb = ld(pers, fc1_b, (1, H), "fc1b", nc.scalar)
            fc2b_sb = ld(pers, fc2_b, (1, H), "fc2b", nc.scalar)

            hnei_sb = [mk(pers, (P, MLK, H), "hnei0", F8),
                       mk(pers, (P, MLK, H), "hnei1", F8)]

            with tc.tile_pool(name="psA", bufs=3, space="PSUM") as psA:
                # ---- h_nei shards: elu(featX @ fcX_w + b) in fp8 -----
                for v, (fT, fw, fb) in enumerate(
                    [(feat1T_sb, fc1w_sb, fc1b_sb),
                     (feat2T_sb, fc2w_sb, fc2b_sb)]
                ):
                    for mc in range(MLK):
                        ps = psA.tile([P, H], FP, tag="psA", name="ps_hnei")
                        nc.tensor.matmul(ps[:], ones_row[:, 0:P], fb[:],
                                         start=True, stop=False)
                        for kp in range(MLK // 2):
                            nc.tensor.matmul(
                                ps[:],
                                fT[:, 2 * kp:2 * kp + 2, mc * P:(mc + 1) * P],
                                fw[:, 2 * kp:2 * kp + 2, :],
                                start=False, stop=(kp == MLK // 2 - 1),
                                perf_mode=DR)
                        q, r = elu(ps[:])
                        nc.vector.tensor_tensor(
                            out=hnei_sb[v][:, mc, :], in0=q[:], in1=r[:],
                            op=ALU.add)

                # ---- h[tar|mask]T + y1_mean first: feeds the early
                # y1_mean AllGather (second collective doorbell) ------
                hthm_sb = mk(pers, (P, HK, 4 * NL), "hthm", BF)
                for hc in range(HK):
                    ps = psA.tile([P, 2 * NL], FP, tag="psA", name="ps_ht")
                    for k in range(D0K):
                        nc.tensor.matmul(
                            ps[:], fc0w_sb[:, k, hc * P:(hc + 1) * P],
                            f0m_sb[:, k, :],
                            start=(k == 0), stop=(k == D0K - 1))
                    q, r = elu(ps[:], ebias=fc0b_sb[:, hc:hc + 1])
                    nc.vector.tensor_tensor(
                        out=hthm_sb[:, hc, 0:2 * NL], in0=q[:], in1=r[:],
                        op=ALU.add)
                htar8 = mk(pers, (P, HK, NL), "htar8", F8)
                nc.vector.tensor_copy(htar8[:], hthm_sb[:, :, 0:NL])
                psm = psA.tile([P, E], FP, tag="psA", name="ps_y1m")
                for kp in range(HK // 2):
                    nc.tensor.matmul(
                        psm[:], htar8[:, 2 * kp:2 * kp + 2, :],
                        gcnw1_sb[:, 2 * kp:2 * kp + 2, :],
                        start=(kp == 0), stop=(kp == HK // 2 - 1),
                        perf_mode=DR)
                stm = wkS.tile([NL, E], F8, tag="stm", name="stm")
                nc.vector.tensor_scalar_mul(stm[:], psm[:], 0.5)
                nc.sync.dma_start(agm_in[:], stm[:])
                for hc in range(HK):
                    nc.vector.tensor_copy(hthm_sb[:, hc, 2 * NL:4 * NL],
                                          hthm_sb[:, hc, 0:2 * NL])

                # ---- partial aggregation (feature-major, fp8) --------
                wq = [nc.sync, nc.scalar]
                for v, neiT in enumerate([nei0T_sb, nei1T_sb]):
                    for hc in range(HK):
                        for jh in range(2):
                            ps = psA.tile([P, 512], FP, tag="psA",
                                          name="ps_pr")
                            for kp in range(MLK // 2):
                                nc.tensor.matmul(
                                    ps[:],
                                    hnei_sb[v][:, 2 * kp:2 * kp + 2,
                                               hc * P:(hc + 1) * P],
                                    neiT[:, 2 * kp:2 * kp + 2,
                                         jh * 512:(jh + 1) * 512],
                                    start=(kp == 0),
                                    stop=(kp == MLK // 2 - 1),
                                    perf_mode=DR)
                            prs = wkE.tile([P, 512], F8, tag="prs",
                                           name="prs")
                            nc.vector.tensor_copy(prs[:], ps[:])
                            wq[(hc + jh) % 2].dma_start(
                                rs_in[jh * 4:(jh + 1) * 4, :,
                                      v * HK + hc:v * HK + hc + 1,
                                      :].rearrange(
                                          "b p one n -> p b (one n)"),
                                prs[:].rearrange("p (b n) -> p b n", b=4))

            # cc stream order: AG(y1_mean) -> RS -> AG(zc) -> AG1 ...
            nc.gpsimd.collective_compute(
                "AllGather", ALU.bypass, replica_groups=RG,
                ins=[agm_in[:].opt()], outs=[agm_out[:].opt()])
            nc.gpsimd.collective_compute(
                "ReduceScatter", ALU.add, replica_groups=RG,
                ins=[rs_in[:].opt()], outs=[rs_out[:].opt()])

            # ============ phase 1 (overlaps the ReduceScatter) ========
            with tc.tile_pool(name="psB", bufs=3, space="PSUM") as psB, \
                 tc.tile_pool(name="psS", bufs=4, space="PSUM") as psS:
                adj0T_sb = ld(pers, adj0T, (P, NB, NL), "adj0T", nc.sync)
                adj1T_sb = ld(pers, adj1T, (P, NB, NL), "adj1T", nc.scalar)
                madj0T_sb = ld(pers, madj0T, (P, NB, NL), "madj0T", nc.sync)
                madj1T_sb = ld(pers, madj1T, (P, NB, NL), "madj1T",
                               nc.scalar)
                mnadjTf_sb = ld(pers, mnadjTf, (P, NB, N), "mnadjTf",
                                nc.sync)
                mnadjTl_sb = ld(pers, mnadjTl, (P, NB, NL), "mnadjTl",
                                nc.scalar)
                agg0w_sb = ld(pers, agg0_w, (P, HK, H), "agg0w", nc.sync)
                agg1w_sb = ld(pers, agg1_w, (P, HK, H), "agg1w", nc.scalar)
                nei0R_sb = ld(pers, nei0R, (P, M), "nei0R", nc.sync)
                nei1R_sb = ld(pers, nei1R, (P, M), "nei1R", nc.scalar)
                gcnw2_sb = ld(pers, gcn_w2, (E, E), "gcnw2", nc.sync)
                attw_sb = ld(pers, att_w, (E, E), "attw", nc.sync)
                projw_sb = ld(pers, proj_w, (E, E), "projw", nc.sync)
                mlp1w_sb = ld(pers, mlp1_w, (E, 16), "mlp1w", nc.sync)
                sel_sb = ld(pers, sel16, (16, 16 * P), "sel", nc.scalar)
                eye_sb = ld(pers, eye128, (P, P), "eye", nc.scalar)
                gcnb1_sb = ld(pers, gcn_b1, (P, 1), "gcnb1", nc.sync)
                gcnb2_sb = ld(pers, gcn_b2, (P, 1), "gcnb2", nc.sync)
                attb_sb = ld(pers, att_b, (E, 1), "attb", nc.sync)
                attv_sb = ld(pers, att_vec, (E, 1), "attv", nc.sync)
                projb_sb = ld(pers, proj_b, (E, 1), "projb", nc.sync)
                b1bc16 = mk(pers, (P, 16), "b1bc16")
                nc.sync.dma_start(b1bc16[:], mlp1_b[:].to_broadcast((P, 16)))
                m2bc = mk(pers, (P, 16), "m2bc")
                nc.sync.dma_start(m2bc[:], mlp2_w[:].to_broadcast((P, 16)))
                b2h = mk(pers, (P, 1), "b2h")
                nc.sync.dma_start(b2h[:], mlp2_b[:].to_broadcast((P, 1)))
                nc.vector.tensor_scalar_mul(b2h[:], b2h[:], 0.5)

                # counts from local nei rows (exact, no collective)
                cnt = [mk(pers, (P, 1), "cnt0"), mk(pers, (P, 1), "cnt1")]
                nc.vector.reduce_sum(cnt[0][:], nei0R_sb[:],
                                     axis=mybir.AxisListType.X)
                nc.vector.reduce_sum(cnt[1][:], nei1R_sb[:],
                                     axis=mybir.AxisListType.X)
                rec4 = []
                for v in range(2):
                    cm = wkS.tile([P, 1], FP, tag="cm", name="cm")
                    nc.vector.tensor_scalar_max(cm[:], cnt[v][:], 1.0)
                    rc = wkS.tile([P, 1], FP, tag="rc", name="rc")
                    nc.vector.reciprocal(rc[:], cm[:])
                    rcb = wkS.tile([P, 1], BF, tag="rcb", name="rcb")
                    nc.vector.tensor_copy(rcb[:], rc[:])
                    pst = psS.tile([1, P], FP, tag="psS", name="ps_rT")
                    nc.tensor.matmul(pst[:], rcb[:], eye_sb[:])
                    rrow4 = wkS.tile([1, 4, P], BF, tag="rrow4",
                                     name="rrow4")
                    for t4 in range(4):
                        nc.vector.tensor_copy(rrow4[:, t4, :], pst[:])
                    psb = psB.tile([P, 4 * P], FP, tag="psB", name="ps_rbc")
                    nc.tensor.matmul(psb[:], ones_row[:, 0:P],
                                     rrow4[:].rearrange("o a b -> o (a b)"))
                    rb = mk(pers, (P, 4 * P), f"rec4_{v}", BF)
                    nc.vector.tensor_copy(rb[:], psb[:])
                    rec4.append(rb)

                # ---- z_coarse chain: full-N mean conv (in RS window) -
                y1m_sb = mk(pers, (P, NB, E), "y1mall", F8)
                nc.sync.dma_start(
                    y1m_sb[:], agm_out[:].rearrange("(b p) e -> p b e", p=P))
                hmT_sb = mk(pers, (E, N), "hmT", F8)
                for jh in range(2):
                    ps = psB.tile([P, 512], FP, tag="psB", name="ps_hm")
                    for bp in range(NB // 2):
                        nc.tensor.matmul(
                            ps[0:E, :], y1m_sb[:, 2 * bp:2 * bp + 2, :],
                            mnadjTf_sb[:, 2 * bp:2 * bp + 2,
                                       jh * 512:(jh + 1) * 512],
                            start=(bp == 0), stop=(bp == NB // 2 - 1),
                            perf_mode=DR)
                    nc.vector.tensor_scalar(
                        out=hmT_sb[:, jh * 512:(jh + 1) * 512],
                        in0=ps[0:E, :], scalar1=gcnb1_sb[0:E, :],
                        scalar2=0.0, op0=ALU.add, op1=ALU.max)
                # y2_mean (x0.5 for the mean-adj sum) [p, NB, E] fp8
                y2m_sb = mk(pers, (P, NB, E), "y2m", F8)
                for b in range(NB):
                    ps = psS.tile([P, E], FP, tag="psS", name="ps_y2m")
                    nc.tensor.matmul(ps[:], hmT_sb[:, b * P:(b + 1) * P],
                                     gcnw2_sb[:])
                    nc.vector.tensor_scalar_mul(y2m_sb[:, b, :], ps[:], 0.5)
                zT_sb = mk(pers, (E, 5, NL), "zT", BF)
                pszm = psS.tile([E, NL], FP, tag="psS", name="ps_zm")
                for bp in range(NB // 2):
                    nc.tensor.matmul(
                        pszm[:], y2m_sb[:, 2 * bp:2 * bp + 2, :],
                        mnadjTl_sb[:, 2 * bp:2 * bp + 2, :],
                        start=(bp == 0), stop=(bp == NB // 2 - 1),
                        perf_mode=DR)
                nc.vector.tensor_scalar_add(zT_sb[:, 0, :], pszm[:],
                                            gcnb2_sb[0:E, :])

                # proj z_coarse + colnorm -> AG(zc)
                psz = psB.tile([P, 512], FP, tag="psB", name="ps_pzc")
                nc.tensor.matmul(psz[0:E, 0:NL], projw_sb[:], zT_sb[:, 0, :])
                tfc = mk(pers, (E, NL), "tfc")
                nc.scalar.activation(tfc[:], psz[0:E, 0:NL], AF.Tanh,
                                     bias=projb_sb[:])
                sqc = wkS.tile([E, NL], BF, tag="sqc", name="sqc")
                nc.vector.tensor_mul(sqc[:], tfc[:], tfc[:])
                pss = psS.tile([1, NL], FP, tag="psS", name="ps_ssc")
                nc.tensor.matmul(pss[:], ones_col[0:E, :], sqc[:])
                rzc = rsqrt_row(pss[:], NL, "rzc")
                psbz = psS.tile([E, NL], FP, tag="psS", name="ps_nbz")
                nc.tensor.matmul(psbz[:], ones_row[:, 0:E], rzc[:])
                zcT_sb = mk(pers, (E, NL), "zcT", BF)
                nc.vector.tensor_mul(zcT_sb[:], tfc[:], psbz[:])
                nc.sync.dma_start(ag3_in[:], zcT_sb[:])
                nc.gpsimd.collective_compute(
                    "AllGather", ALU.bypass, replica_groups=RG,
                    ins=[ag3_in[:].opt()], outs=[ag3_out[:].opt()])

                # ================= post-RS: views + fine GCN ==========
                aggT_sb = mk(pers, (P, 2 * HK, NL), "aggT", F8)
                nc.sync.dma_start(aggT_sb[:], rs_out[:])
                aggS_sb = mk(pers, (P, 2 * HK, 2 * NL), "aggS", F8)
                for v in range(2):
                    for half in range(2):
                        nc.vector.tensor_tensor(
                            out=aggS_sb[:, v * HK:(v + 1) * HK,
                                        half * NL:(half + 1) * NL],
                            in0=aggT_sb[:, v * HK:(v + 1) * HK, :],
                            in1=rec4[v][:].rearrange(
                                "p (a b) -> p a b", a=HK),
                            op=ALU.mult)

                # both views + masks in one [P, 512] pass per h-chunk:
                # cols [v0tar | v0mask | v1tar | v1mask]
                xs4 = mk(pers, (P, HK, 4 * NL), "xs4", F8)
                for hc in range(HK):
                    ps = psB.tile([P, 4 * NL], FP, tag="psB", name="ps_x2")
                    for v, aggw in enumerate([agg0w_sb, agg1w_sb]):
                        half = ps[:, v * 2 * NL:(v + 1) * 2 * NL]
                        nc.tensor.matmul(
                            half, eye_sb[:],
                            hthm_sb[:, hc, v * 2 * NL:(v + 1) * 2 * NL],
                            start=True, stop=False)
                        for kp in range(HK // 2):
                            nc.tensor.matmul(
                                half,
                                aggw[:, 2 * kp:2 * kp + 2,
                                     hc * P:(hc + 1) * P],
                                aggS_sb[:, v * HK + 2 * kp:
                                        v * HK + 2 * kp + 2, :],
                                start=False, stop=(kp == HK // 2 - 1),
                                perf_mode=DR)
                    q, r = elu(ps[:])
                    nc.vector.tensor_tensor(
                        out=xs4[:, hc, :], in0=q[:], in1=r[:], op=ALU.add)

                # GCN layer-1 linear; st4a cols [v0, v1, m0, m1]
                st4a = mk(pers, (NL, 4, E), "st4a", F8)
                for c0, slot in [(0, 0), (2 * NL, 1), (NL, 2), (3 * NL, 3)]:
                    ps = psS.tile([P, E], FP, tag="psS", name="ps_y1")
                    for kp in range(HK // 2):
                        nc.tensor.matmul(
                            ps[:], xs4[:, 2 * kp:2 * kp + 2, c0:c0 + NL],
                            gcnw1_sb[:, 2 * kp:2 * kp + 2, :],
                            start=(kp == 0), stop=(kp == HK // 2 - 1),
                            perf_mode=DR)
                    nc.vector.tensor_copy(st4a[:, slot, :], ps[:])
                nc.sync.dma_start(
                    ag1_in[:].rearrange("n (g e) -> n g e", g=4), st4a[:])
                nc.gpsimd.collective_compute(
                    "AllGather", ALU.bypass, replica_groups=RG,
                    ins=[ag1_in[:].opt()], outs=[ag1_out[:].opt()])

                def conv_fine(y_sb, badd, relu, outs):
                    """4 fine graph convs; y_sb [P, NB, 4E] fp8 cols
                    [v0, v1, m0, m1]; outs: list of 4 (dst_ap)."""
                    pp = [psS.tile([E, NL], FP, tag="psS", name=f"pc{g}")
                          for g in range(4)]
                    adjs = [adj0T_sb, adj1T_sb, madj0T_sb, madj1T_sb]
                    for bp in range(NB // 2):
                        for g in range(4):
                            nc.tensor.matmul(
                                pp[g][:],
                                y_sb[:, 2 * bp:2 * bp + 2,
                                     g * E:(g + 1) * E],
                                adjs[g][:, 2 * bp:2 * bp + 2, :],
                                start=(bp == 0), stop=(bp == NB // 2 - 1),
                                perf_mode=DR)
                    op1 = ALU.max if relu else ALU.bypass
                    for g in range(4):
                        nc.vector.tensor_scalar(
                            out=outs[g], in0=pp[g][:],
                            scalar1=badd[0:E, :], scalar2=0.0,
                            op0=ALU.add, op1=op1)

                y1_sb = mk(pers, (P, NB, 4 * E), "y1", F8)
                nc.sync.dma_start(
                    y1_sb[:], ag1_out[:].rearrange("(b p) f -> p b f", p=P))
                h4_sb = mk(pers, (E, 4, NL), "h4", BF)
                conv_fine(y1_sb, gcnb1_sb, True,
                          [h4_sb[:, g, :] for g in range(4)])
                st4b = mk(pers, (NL, 4, E), "st4b", F8)
                for g in range(4):
                    ps = psS.tile([P, E], FP, tag="psS", name="ps_y2")
                    nc.tensor.matmul(ps[:], h4_sb[:, g, :], gcnw2_sb[:])
                    nc.vector.tensor_copy(st4b[:, g, :], ps[:])
                nc.sync.dma_start(
                    ag2_in[:].rearrange("n (g e) -> n g e", g=4), st4b[:])
                nc.gpsimd.collective_compute(
                    "AllGather", ALU.bypass, replica_groups=RG,
                    ins=[ag2_in[:].opt()], outs=[ag2_out[:].opt()])

                # zcall + BT as soon as AG(zc) lands
                zcall_sb = mk(pers, (E, C, P), "zcall", BF)
                zsrc = ag3_out[:].rearrange("(s p) n -> p s n", p=E)
                nc.sync.dma_start(zcall_sb[:, 0:4, :], zsrc[:, 0:4, :])
                nc.scalar.dma_start(zcall_sb[:, 4:8, :], zsrc[:, 4:8, :])
                BT_sb = mk(pers, (16, N), "BT", BF)
                for jh in range(2):
                    pbt = psB.tile([P, 512], FP, tag="psB", name="ps_BT")
                    nc.tensor.matmul(pbt[0:16, 0:512], mlp1w_sb[:],
                                     zcall_sb[:, jh * 4:(jh + 1) * 4, :])
                    nc.vector.tensor_copy(
                        BT_sb[:, jh * 512:(jh + 1) * 512], pbt[0:16, 0:512])

                y2_sb = mk(pers, (P, NB, 4 * E), "y2", F8)
                nc.sync.dma_start(
                    y2_sb[:], ag2_out[:].rearrange("(b p) f -> p b f", p=P))
                # conv2 -> zT slots [mean, v0, m0, v1, m1]
                conv_fine(y2_sb, gcnb2_sb, False,
                          [zT_sb[:, 1, :], zT_sb[:, 3, :],
                           zT_sb[:, 2, :], zT_sb[:, 4, :]])

                # ---- attention (scale-after-matmul) -> AG3a ----------
                z4 = zT_sb[:, 1:5, :]
                psa4 = psB.tile([P, 512], FP, tag="psB", name="ps_att4")
                nc.tensor.matmul(psa4[0:E, 0:4 * NL], attw_sb[:], z4)
                sq4 = wkS.tile([E, 4 * NL], BF, tag="sq4", name="sq4")
                nc.vector.tensor_mul(sq4[:], z4, z4)
                psn4 = psB.tile([P, 512], FP, tag="psB", name="ps_n4")
                nc.tensor.matmul(psn4[0:1, 0:4 * NL], ones_col[0:E, :],
                                 sq4[:])
                rn4 = rsqrt_row(psn4[0:1, 0:4 * NL], 4 * NL, "rn4")
                psb4 = psB.tile([P, 512], FP, tag="psB", name="ps_nb4")
                nc.tensor.matmul(psb4[0:E, 0:4 * NL], ones_row[:, 0:E],
                                 rn4[:])
                rn4bc = wkS.tile([E, 4 * NL], FP, tag="rn4bc", name="rn4bc")
                nc.vector.tensor_copy(rn4bc[:], psb4[0:E, 0:4 * NL])
                ta4in = wkS.tile([E, 4 * NL], FP, tag="ta4in", name="ta4in")
                nc.vector.tensor_mul(ta4in[:], psa4[0:E, 0:4 * NL],
                                     rn4bc[:])
                ta4 = wkS.tile([E, 4 * NL], BF, tag="ta4", name="ta4")
                nc.scalar.activation(ta4[:], ta4in[:], AF.Tanh,
                                     bias=attb_sb[:])
                pse = psB.tile([P, 512], FP, tag="psB", name="ps_e")
                nc.tensor.matmul(pse[0:1, 0:4 * NL], attv_sb[:], ta4[:])
                er4 = wkS.tile([1, 4], FP, tag="er4", name="er4")
                nc.vector.reduce_sum(
                    er4[:],
                    pse[0:1, 0:4 * NL].rearrange("o (v n) -> o v n", v=4),
                    axis=mybir.AxisListType.X)
                e_row = wkS.tile([1, P], BF, tag="e_row", name="e_row")
                nc.vector.memset(e_row[:], 0.0)
                nc.vector.tensor_scalar_mul(e_row[:, 0:4], er4[:], 1.0 / N)
                nc.sync.dma_start(ag3a_in[:], e_row[:])
                nc.gpsimd.collective_compute(
                    "AllGather", ALU.bypass, replica_groups=RG,
                    ins=[ag3a_in[:].opt()], outs=[ag3a_out[:].opt()])

                # hs + per-view projections (overlap the AG3a wait)
                hsT_sb = mk(pers, (E, 4, NL), "hsT", BF)
                nc.vector.tensor_mul(hsT_sb[:], z4, rn4bc[:])
                pj4 = psB.tile([P, 512], FP, tag="psB", name="ps_pj4")
                nc.tensor.matmul(pj4[0:E, 0:4 * NL], projw_sb[:],
                                 hsT_sb[:])
                pj4b = wkS.tile([E, 4 * NL], FP, tag="pj4b", name="pj4b")
                nc.vector.tensor_copy(pj4b[:], pj4[0:E, 0:4 * NL])

                # ---- softmax over views; z_fine; proj; A; diag -------
                e8_sb = wkS.tile([C, 4], BF, tag="e8", name="e8")
                nc.sync.dma_start(e8_sb[:], ag3a_out[:, 0:4])
                pse2 = psS.tile([1, 4], FP, tag="psS", name="ps_e2")
                nc.tensor.matmul(pse2[:], ones_col[0:C, :], e8_sb[:])
                ee = wkS.tile([1, 4], FP, tag="ee", name="ee")
                nc.scalar.activation(ee[:], pse2[:], AF.Exp)
                se = wkS.tile([1, 1], FP, tag="se", name="se")
                nc.vector.reduce_sum(se[:], ee[:], axis=mybir.AxisListType.X)
                nc.vector.reciprocal(se[:], se[:])
                beta_row = wkS.tile([1, 4], BF, tag="beta", name="beta")
                nc.vector.tensor_scalar_mul(beta_row[:], ee[:], se[:])
                psbb = psS.tile([E, 4], FP, tag="psS", name="ps_beta")
                nc.tensor.matmul(psbb[:], ones_row[:, 0:E], beta_row[:])
                beta_bc = wkS.tile([E, 4], FP, tag="beta_bc",
                                   name="beta_bc")
                nc.vector.tensor_copy(beta_bc[:], psbb[:])

                zfp = wkS.tile([E, NL], FP, tag="zfp", name="zfp")
                nc.vector.tensor_scalar(
                    out=zfp[:], in0=pj4b[:, 0:NL], scalar1=beta_bc[:, 0:1],
                    scalar2=0.0, op0=ALU.mult, op1=ALU.add)
                for v in range(1, 4):
                    nc.vector.scalar_tensor_tensor(
                        out=zfp[:], in0=pj4b[:, v * NL:(v + 1) * NL],
                        scalar=beta_bc[:, v:v + 1], in1=zfp[:],
                        op0=ALU.mult, op1=ALU.add)
                tf2 = mk(pers, (E, NL), "tf2", BF)
                nc.scalar.activation(tf2[:], zfp[:], AF.Tanh,
                                     bias=projb_sb[:])
                sqf = wkS.tile([E, NL], BF, tag="sqf", name="sqf")
                nc.vector.tensor_mul(sqf[:], tf2[:], tf2[:])
                pssf = psS.tile([1, NL], FP, tag="psS", name="ps_ssf")
                nc.tensor.matmul(pssf[:], ones_col[0:E, :], sqf[:])
                rzf = rsqrt_row(pssf[:], NL, "rzf")
                psbf = psS.tile([E, NL], FP, tag="psS", name="ps_nbf")
                nc.tensor.matmul(psbf[:], ones_row[:, 0:E], rzf[:])
                zfn_bf = mk(pers, (E, NL), "zfn_bf", BF)
                nc.vector.tensor_mul(zfn_bf[:], tf2[:], psbf[:])

                # A = zfn @ mlp1_w + b1  [NL, 16]
                psa2 = psS.tile([NL, 16], FP, tag="psS", name="ps_A")
                nc.tensor.matmul(psa2[:], zfn_bf[:], mlp1w_sb[:])
                A_sb = mk(pers, (NL, 16), "A")
                nc.vector.tensor_add(A_sb[:], psa2[:], b1bc16[:, 0:16])

                # diag = (zfn.zc)/tau
                prod = wkS.tile([E, NL], BF, tag="prod", name="prod")
                nc.vector.tensor_mul(prod[:], zfn_bf[:], zcT_sb[:])
                psd = psS.tile([NL, 1], FP, tag="psS", name="ps_diag")
                nc.tensor.matmul(psd[:], prod[:], ones_col[0:E, :])
                diag_sb = mk(pers, (NL, 1), "diag")
                nc.vector.tensor_scalar_mul(diag_sb[:], psd[:], 1.0 / TAU)

            # ================= InfoNCE tail (2-bank psum) =============
            with tc.tile_pool(name="psT", bufs=3, space="PSUM") as psT:
                # dots = exp((zfn.zc) / tau)  (exp/tanh table)
                psl = psT.tile([P, N], FP, tag="psT", name="ps_log")
                for jh in range(2):
                    nc.tensor.matmul(
                        psl[:, jh * 512:(jh + 1) * 512], zfn_bf[:],
                        zcall_sb[:, jh * 4:(jh + 1) * 4, :])
                dots_sb = mk(pers, (P, N), "dots", BF)
                nc.scalar.activation(dots_sb[:], psl[:], AF.Exp,
                                     scale=1.0 / TAU)

                # acc = sum_h tanh(A[:,h] + B[j,h]) * m2[h]
                accv = mk(pers, (P, N), "accv", BF)
                accg = mk(pers, (P, N), "accg", BF)
                for h in range(16):
                    psbt = psT.tile([P, N], FP, tag="psT", name="ps_bbc")
                    for jh in range(2):
                        nc.tensor.matmul(
                            psbt[:, jh * 512:(jh + 1) * 512],
                            sel_sb[:, h * P:(h + 1) * P],
                            BT_sb[:, jh * 512:(jh + 1) * 512])
                    th = wkT.tile([P, N], BF, tag="th", name="th")
                    nc.scalar.activation(th[:], psbt[:], AF.Tanh,
                                         bias=A_sb[:, h:h + 1])
                    acc = accg if h % 2 == 1 else accv
                    if h < 2:
                        nc.vector.tensor_scalar(
                            out=acc[:], in0=th[:], scalar1=m2bc[:, h:h + 1],
                            scalar2=0.0, op0=ALU.mult, op1=ALU.add)
                    else:
                        nc.vector.scalar_tensor_tensor(
                            out=acc[:], in0=th[:], scalar=m2bc[:, h:h + 1],
                            in1=acc[:], op0=ALU.mult, op1=ALU.add)
                nc.vector.tensor_add(accv[:], accv[:], accg[:])

                # sigmoid(x) = 0.5 + 0.5*tanh(x/2): stay on the exp table
                wt_sb = wkT.tile([P, N], BF, tag="wt", name="wt")
                nc.scalar.activation(wt_sb[:], accv[:], AF.Tanh,
                                     scale=0.5, bias=b2h[:])
                w_sb = mk(pers, (P, N), "w", BF)
                nc.vector.tensor_scalar(
                    out=w_sb[:], in0=wt_sb[:], scalar1=0.5, scalar2=0.5,
                    op0=ALU.mult, op1=ALU.add)

                # denom = sum_j dots * w
                denom_sb = wkS.tile([P, 1], FP, tag="denom", name="denom")
                scr = wkT.tile([P, N], BF, tag="scr", name="scr")
                nc.vector.scalar_tensor_tensor(
                    out=scr[:], in0=dots_sb[:], scalar=1.0, in1=w_sb[:],
                    op0=ALU.bypass, op1=ALU.mult, accum_out=denom_sb[:])

                outt = wkS.tile([NL, 2], FP, tag="outt", name="outt")
                nc.vector.tensor_copy(outt[:, 0:1], denom_sb[:])
                nc.vector.tensor_copy(outt[:, 1:2], diag_sb[:])
                nc.sync.dma_start(out_ext[:], outt[:])

    nc.finalize()
    return nc


_NC_CACHE = {}


def _get_nc():
    if "nc" not in _NC_CACHE:
        _NC_CACHE["nc"] = _build()
    return _NC_CACHE["nc"]


def _part3(x, p=128):
    """[(o p), f] row-major -> [p, o*f] (partition-inner layout)."""
    o = x.shape[0] // p
    return x.reshape(o, p, x.shape[1]).transpose(1, 0, 2).reshape(p, -1)


def kernel(**inputs):
    inp = {k: np.ascontiguousarray(np.asarray(v, dtype=np.float32))
           for k, v in inputs.items()}
    nc = _get_nc()

    def bf(x):
        return np.ascontiguousarray(x.astype(NPBF))

    def f8(x):
        return np.ascontiguousarray(x.astype(NPF8))

    rep = {}
    rep["fc0_w"] = bf(_part3(inp["fc0_w"]))
    rep["fc1_w"] = f8(_part3(inp["fc1_w"]))
    rep["fc2_w"] = f8(_part3(inp["fc2_w"]))
    rep["agg0_w"] = f8(_part3(inp["agg0_w"]))
    rep["agg1_w"] = f8(_part3(inp["agg1_w"]))
    rep["gcn_w1"] = f8(_part3(inp["gcn_w1"]))
    for k in ["gcn_w2", "att_w", "proj_w", "mlp1_w"]:
        rep[k] = bf(inp[k])
    rep["sel16"] = bf(
        np.kron(np.eye(16, dtype=np.float32), np.ones((1, P), np.float32)))
    rep["eye128"] = bf(np.eye(P, dtype=np.float32))
    rep["fc0_b"] = np.ascontiguousarray(
        inp["fc0_b"].reshape(HK, P).T)                     # [p, hc]
    rep["fc1_b"] = bf(inp["fc1_b"].reshape(1, H))
    rep["fc2_b"] = bf(inp["fc2_b"].reshape(1, H))
    rep["gcn_b1"] = np.ascontiguousarray(
        np.tile(inp["gcn_b1"].reshape(E), 2).reshape(P, 1))
    rep["gcn_b2"] = np.ascontiguousarray(
        np.tile(inp["gcn_b2"].reshape(E), 2).reshape(P, 1))
    for k in ["att_b", "proj_b"]:
        rep[k] = np.ascontiguousarray(inp[k].reshape(E, 1))
    rep["att_vec"] = bf(inp["att_vec"].reshape(E, 1))
    rep["mlp1_b"] = np.ascontiguousarray(inp["mlp1_b"].reshape(1, 16))
    rep["mlp2_w"] = np.ascontiguousarray(inp["mlp2_w"].reshape(16, 1).T)
    rep["mlp2_b"] = np.ascontiguousarray(inp["mlp2_b"].reshape(1, 1))

    mnadj = inp["adj0"] + inp["adj1"]
    in_maps = []
    for r in range(C):
        rs = slice(r * NL, (r + 1) * NL)
        ms = slice(r * ML, (r + 1) * ML)
        d = dict(rep)
        d["feat1T"] = f8(_part3(inp["feat1"][ms].T))
        d["feat2T"] = f8(_part3(inp["feat2"][ms].T))
        d["nei0T"] = f8(_part3(inp["nei0"][:, ms].T))
        d["nei1T"] = f8(_part3(inp["nei1"][:, ms].T))
        d["nei0R"] = bf(inp["nei0"][rs])
        d["nei1R"] = bf(inp["nei1"][rs])
        d["f0m"] = bf(_part3(np.concatenate(
            [inp["feat0"][rs].T, inp["mask_feat"][rs].T], axis=1)))
        d["adj0T"] = f8(_part3(inp["adj0"][rs].T))
        d["adj1T"] = f8(_part3(inp["adj1"][rs].T))
        d["madj0T"] = f8(_part3(inp["madj0"][rs].T))
        d["madj1T"] = f8(_part3(inp["madj1"][rs].T))
        d["mnadjTf"] = f8(_part3(mnadj.T))
        d["mnadjTl"] = f8(_part3(mnadj[rs].T))
        in_maps.append(d)

    trace = bool(int(os.environ.get("KERNEL_TRACE", "0")))
    res = run_bass_kernel_spmd(
        nc, in_maps, core_ids=list(range(C)), trace=trace)
    if trace:
        _NC_CACHE["exec_time_ns"] = res.exec_time_ns
        _NC_CACHE["trace"] = res.instructions_and_trace
    total = 0.0
    for r in range(C):
        o = np.asarray(res.results[r]["out"], dtype=np.float64)
        total += float(np.sum(np.log(o[:, 0]) - o[:, 1]))
    return np.float32(total / N)
